# revision 11
# baseline (speedup 1.0000x reference)
"""MultiHeadDiffAttn Trainium2 kernel (v6, per-half-head PSUM/es tiles).

Sharding: 8 cores = 4-way data parallel over batch x 2-way tensor parallel
over heads (8 v-heads / 16 half-heads per core).  Each core computes its
batch's qkv projection restricted to its head group, differential attention
with per-half-head softmax, head RMS norm, and a partial output projection
(its 512 rows of w_proj).  Host sums the two partial projections per batch.

Measured device behavior this kernel is shaped around:
  - fp32 matmul streams at ~1/4 the 16-bit rate, so all matmul operands are
    fp16 (PSUM accumulation stays fp32).
  - K<128 16-bit matmuls stream at 2 cycles/col, so the S^T matmuls pad the
    contraction to K=128 via the zero-padded qTp buffer.
  - ACT exp is the attention-phase bottleneck (~0.83ns/col + ~200ns/op).
    Each half-head keeps its OWN S-psum tile and its own es tile: sharing
    one [128,2,T] tile between the pair made S(s+1) wait on BOTH exps of
    iteration s (the dep tracker does not split the halves), serializing
    the loop (v3/v5 lost 30-45us to that).  S matmuls are emitted e-outer
    so exp(e0) starts after half the S work; the causal mask runs per
    half-head on GpSimd; the diagonal AV is emitted LAST per (s,e) group
    so the mask latency hides behind the off-diagonal AVs.
  - 12 s-iterations (all of head 0 + head 1's first 4) are prebaked into
    the tensor-bound qkv phase so ACT works from ~20us: cc order
    0,4,1,5,2,6,3,7 makes the needed chunks finish first, and the head-1
    prebakes interleave between v-projection t-blocks.
  - qTp scatter stays on DVE ([32,*] ops on GpSimd use 2 of 8 Q7 cores and
    run ~4x slow — v4 lost 60us to that).
  - RMS rstd = rsqrt(ms+eps) is computed entirely on DVE (bit-trick seed +
    2 Newton steps on int32/f32 bitcast views): keeping Sqrt off ACT avoids
    the Exp<->Sqrt activation-table thrash (~1.3us per reload).
  - Epilogue runs wide: batched reciprocals [128,4] over the den columns,
    lambda folded via the scalar slot, bank-wide [128,4,64] ops with
    stride-0 broadcast APs, one outcat scale op per head.  RMS in three
    batches (0-3 / 4-6 / 7) so epilogue overlaps attention.
  - y is stored f16; the tail yt copies run on the (idle-by-then) ACT.
"""

import math
from contextlib import ExitStack

import numpy as np

import concourse.bass as bass
import concourse.tile as tile
from concourse import masks, mybir
from concourse.bass_utils import run_bass_kernel_spmd

# The deployed walrus rejects instructions carrying more than one sync wait
# ("Too many sync wait commands" in setupSyncWait).  Legalize at the BIR-JSON
# level: for every instruction with >1 wait, hoist the extra waits onto NoOp
# instructions inserted just before it on the same engine (engine streams are
# in-order, so semantics are identical).
_MAX_WAITS = 1


def _legalize_sync_waits(d):
    for f in d.get("functions", []):
        for bb in f.get("blocks", []):
            out = []
            for inst in bb["instructions"]:
                si = inst.get("sync_info")
                waits = (si or {}).get("on_wait") or []
                if len(waits) > _MAX_WAITS:
                    extra = waits[: len(waits) - _MAX_WAITS]
                    keep = waits[len(waits) - _MAX_WAITS :]
                    for j in range(0, len(extra), _MAX_WAITS):
                        nop = {
                            "engine": inst["engine"],
                            "ins": [],
                            "outs": [],
                            "name": f"{inst['name']}-lw{j}",
                            "opcode": "NoOp",
                            "sync_info": {
                                "on_wait": extra[j : j + _MAX_WAITS],
                                "on_update": [],
                            },
                        }
                        if "debug" in inst:
                            nop["debug"] = inst["debug"]
                        out.append(nop)
                    si["on_wait"] = keep
                out.append(inst)
            bb["instructions"] = out
    return d


_orig_to_json_bytes = bass.Bass.to_json_bytes


def _patched_to_json_bytes(self, *a, **kw):
    import json as _json

    raw = _orig_to_json_bytes(self, *a, **kw)
    return _json.dumps(_legalize_sync_waits(_json.loads(raw))).encode()


bass.Bass.to_json_bytes = _patched_to_json_bytes

F32 = mybir.dt.float32
F16 = mybir.dt.float16
I32 = mybir.dt.int32

B, T, C = 4, 1024, 1024
H_TOT = 16  # total v-heads
HD = 32  # half-head dim
DV = 64  # v-head dim
G = 2  # head groups (tensor parallel)
HPG = H_TOT // G  # 8 v-heads per core
COLS = 1024  # q cols + k cols per group
LAMBDA_INIT = 0.8 - 0.6 * math.exp(-0.3 * (1 - 1))  # 0.2
EPS = 1e-5
N_CORES = 8

NT = T // 128  # 8 t-tiles
NKC = C // 128  # 8 contraction chunks
RSQRT_MAGIC = 0x5F3759DF


def _bcast(ap, n):
    """[128, m] -> [128, m, n] with stride-0 last dim."""
    return ap.unsqueeze(2).broadcast_to([ap.shape[0], ap.shape[1], n])


def _emit(ctx: ExitStack, tc: tile.TileContext, xT, w_qk, w_v, w_p, lam, y):
    nc = tc.nc
    AluOp = mybir.AluOpType
    Act = mybir.ActivationFunctionType

    const = ctx.enter_context(tc.tile_pool(name="const", bufs=1))
    ident = const.tile([128, 128], F16)
    masks.make_identity(nc, ident[:])
    lam_sb = const.tile([128, 1], F32)
    nc.sync.dma_start(out=lam_sb, in_=lam[:])
    magic_sb = const.tile([128, 1], I32)
    nc.vector.memset(magic_sb, RSQRT_MAGIC)
    c1p5_sb = const.tile([128, 1], F32)
    nc.vector.memset(c1p5_sb, 1.5)

    big = ctx.enter_context(tc.tile_pool(name="big", bufs=1))
    qkT_sb = big.tile([128, 8, T], F16)  # row-chunks of [COLS, T]
    v_sb = big.tile([128, NT, HPG, 128], F16)  # [s-chunk][head][dv | ones | 0-pad]
    wp_sb = big.tile([128, 4, C], F16)
    # per-half-head q, zero-padded to K=128: data lives at the same 32-row
    # strip as that half-head's k rows inside its qkT chunk, so the S^T
    # matmul can contract over the full 128 partitions at full stream rate
    # (the other half-heads' k rows meet zero q rows).
    qTp_sb = big.tile([128, 2 * HPG, T], F16)

    # zero qTp per half-head group so each scatter only waits on its own
    # memset; then the v padding
    for cc in range(4):
        nc.gpsimd.memset(qTp_sb[:, 4 * cc : 4 * cc + 4, :], 0.0)
    nc.gpsimd.memset(v_sb[:, :, :, DV + 1 :], 0.0)
    es_pool = ctx.enter_context(tc.tile_pool(name="es", bufs=19))

    prebaked = {}  # h -> list of (s, es2, chunks)

    # ---------------- phase 1+2: qkv projections + prebaked attention ----------------
    with (
        tc.tile_pool(name="xw", bufs=1) as xw,
        tc.tile_pool(name="mmps", bufs=3, space="PSUM") as mmps,
        tc.tile_pool(name="spre", bufs=1, space="PSUM") as spre,
    ):
        xT_sb = xw.tile([128, NKC, T], F16)
        wqk_sb = xw.tile([128, NKC, COLS], F16)
        wv_sb = xw.tile([128, NKC, 512], F16)

        xT_r = xT[:].rearrange("(c p) t -> p c t", p=128)
        wqk_r = w_qk[:].rearrange("(c p) m -> p c m", p=128)

        def load_wqk(cc):
            nc.sync.dma_start(
                out=wqk_sb[:, :, cc * 128 : (cc + 1) * 128],
                in_=wqk_r[:, :, cc * 128 : (cc + 1) * 128],
            )

        # load order: interleave per-kc pieces of wqk0 and xT so the first
        # matmul group starts after ~160KB instead of ~1.3MB
        for kc in range(NKC):
            nc.sync.dma_start(
                out=wqk_sb[:, kc, 0:128], in_=wqk_r[:, kc, 0:128]
            )
            nc.sync.dma_start(
                out=xT_sb[:, kc, 0:512], in_=xT_r[:, kc, 0:512]
            )
        for kc in range(NKC):
            nc.sync.dma_start(
                out=xT_sb[:, kc, 512:1024], in_=xT_r[:, kc, 512:1024]
            )
        load_wqk(4)
        load_wqk(1)
        nc.sync.dma_start(out=wv_sb, in_=w_v[:].rearrange("(c p) m -> p c m", p=128))
        for cc in (5, 2, 6, 3, 7):
            load_wqk(cc)
        nc.sync.dma_start(
            out=wp_sb, in_=w_p[:].rearrange("(c p) m -> p c m", p=128)
        )

        def emit_prebake(h, s):
            t0 = 128 * s
            chunks = [(t0, 512), (512, 1024)] if s < 4 else [(t0, 1024)]
            s2 = [
                spre.tile([128, T], F32, tag=f"spre{e}", name=f"sp{e}_{h}_{s}")
                for e in range(2)
            ]
            es2 = [
                es_pool.tile([128, T], F16, tag=f"es{e}", name=f"esp{e}_{h}_{s}")
                for e in range(2)
            ]
            for e in range(2):
                for c0, c1 in chunks:
                    nc.tensor.matmul(
                        s2[e][:, c0:c1],
                        qkT_sb[:, 4 + h // 2, t0 : t0 + 128],
                        qTp_sb[:, 2 * h + e, c0:c1],
                        start=True,
                        stop=True,
                    )
                nc.scalar.activation(
                    out=es2[e][:, t0:T],
                    in_=s2[e][:, t0:T],
                    func=Act.Exp,
                    scale=1.0 / 32.0,
                )
                nc.gpsimd.affine_select(
                    out=es2[e][:, t0 : t0 + 128],
                    in_=es2[e][:, t0 : t0 + 128],
                    pattern=[[1, 128]],
                    compare_op=AluOp.is_ge,
                    fill=0.0,
                    base=0,
                    channel_multiplier=-1,
                )
            prebaked.setdefault(h, []).append((s, es2, chunks))

        # qkT[cc-block, :] = w_qk[:, cc-block].T @ x^T
        # Interleaved cc order so the prebakes (need the cc=0/1 scatters
        # + the cc=4 kT chunk) can start early.
        cc_order = [0, 4, 1, 5, 2, 6, 3, 7]
        prebake_after = {1: (0, 0), 2: (0, 1), 3: (0, 2), 4: (0, 3),
                         5: (0, 4), 6: (0, 5), 7: (0, 6)}  # h0 s7 after loop
        for pos, cc in enumerate(cc_order):
            for nh in range(2):
                ps = mmps.tile([128, 512], F32, tag="mmps", name=f"qk{cc}{nh}")
                for kc in range(NKC):
                    nc.tensor.matmul(
                        ps,
                        wqk_sb[:, kc, cc * 128 : (cc + 1) * 128],
                        xT_sb[:, kc, nh * 512 : (nh + 1) * 512],
                        start=(kc == 0),
                        stop=(kc == NKC - 1),
                    )
                nc.vector.tensor_copy(
                    out=qkT_sb[:, cc, nh * 512 : (nh + 1) * 512], in_=ps
                )
                if cc < 4:  # q chunk: scatter the 4 half-heads into qTp
                    for j in range(4):
                        hh = cc * 4 + j
                        nc.vector.tensor_copy(
                            out=qTp_sb[
                                j * 32 : (j + 1) * 32,
                                hh,
                                nh * 512 : (nh + 1) * 512,
                            ],
                            in_=qkT_sb[
                                j * 32 : (j + 1) * 32,
                                cc,
                                nh * 512 : (nh + 1) * 512,
                            ],
                        )
            if pos in prebake_after:
                emit_prebake(*prebake_after[pos])
        emit_prebake(0, 7)

        # v[t-block, :] = x @ w_v ; scatter heads into v_sb, slot 64 = ones;
        # head-1 prebakes interleave between the t-blocks
        for tt in range(NT):
            ps = mmps.tile([128, 512], F32, tag="mmps", name=f"v{tt}")
            for kc in range(NKC):
                nc.tensor.matmul(
                    ps,
                    xT_sb[:, kc, tt * 128 : (tt + 1) * 128],
                    wv_sb[:, kc, :],
                    start=(kc == 0),
                    stop=(kc == NKC - 1),
                )
            nc.vector.tensor_copy(
                out=v_sb[:, tt, :, 0:DV],
                in_=ps[:].rearrange("p (h d) -> p h d", h=HPG),
            )
            nc.vector.memset(v_sb[:, tt, :, DV : DV + 1], 1.0)
            emit_prebake(1, tt)

    mid = ctx.enter_context(tc.tile_pool(name="mid", bufs=1))
    outcat_sb = mid.tile([128, NT, HPG * DV], F16)  # [t-chunk][512]
    outcatT_sb = mid.tile([128, 4, T], F16)  # row-chunks of [512, T]

    # ---------------- phase 3: differential attention ----------------
    with (
        tc.tile_pool(name="sps", bufs=1, space="PSUM") as s_pool,
        tc.tile_pool(name="ups", bufs=1, space="PSUM") as u_pool,
        tc.tile_pool(name="comb", bufs=6) as comb,
        tc.tile_pool(name="ohp", bufs=1, space="SBUF") as ohp,
    ):
        oh_all = ohp.tile([128, HPG, NT, DV], F32, tag="ohall", name="ohall")
        ssq_all = ohp.tile([128, HPG * NT], F32, tag="ssqall", name="ssqall")

        # first live head's opening iterations, emitted around the prebaked
        # heads' AV dumps so ACT has work while the dumps drain
        h_live = max(prebaked.keys()) + 1
        live_s_tiles = [
            s_pool.tile([128, T], F32, tag=f"s{e}", name=f"s{e}_{h_live}")
            for e in range(2)
        ]

        def emit_live_iter(s):
            t0 = 128 * s
            chunks = [(t0, 512), (512, 1024)] if s < 4 else [(t0, 1024)]
            es2 = [
                es_pool.tile(
                    [128, T], F16, tag=f"es{e}", name=f"es{e}_{h_live}_{s}"
                )
                for e in range(2)
            ]
            for e in range(2):
                for c0, c1 in chunks:
                    nc.tensor.matmul(
                        live_s_tiles[e][:, c0:c1],
                        qkT_sb[:, 4 + h_live // 2, t0 : t0 + 128],
                        qTp_sb[:, 2 * h_live + e, c0:c1],
                        start=True,
                        stop=True,
                    )
                nc.scalar.activation(
                    out=es2[e][:, t0:T],
                    in_=live_s_tiles[e][:, t0:T],
                    func=Act.Exp,
                    scale=1.0 / 32.0,
                )
                nc.gpsimd.affine_select(
                    out=es2[e][:, t0 : t0 + 128],
                    in_=es2[e][:, t0 : t0 + 128],
                    pattern=[[1, 128]],
                    compare_op=AluOp.is_ge,
                    fill=0.0,
                    base=0,
                    channel_multiplier=-1,
                )
            prebaked.setdefault(h_live, []).append((s, es2, chunks))

        emit_live_iter(0)

        def emit_av(h, s, e, es_e, chunks):
            # U[t-block, dv|den] += expS^T[s-block, t-block].T @ v_aug[s-block]
            # diag tj last so the fused-mask latency hides behind the others;
            # start fires on the first tj EMITTED for each bank at s==0
            tjs = list(range(s + 1, NT)) + [s]
            for tj in tjs:
                off = (tj % 4) * 128
                nc.tensor.matmul(
                    u_tiles[e][tj // 4][:, off : off + DV + 1],
                    es_e[:, tj * 128 : (tj + 1) * 128],
                    v_sb[:, s, h, 0 : DV + 1],
                    start=(s == 0 and tj in (1, 4)),
                    stop=(s == tj and tj % 4 == 3),
                )

        # ---- RMS in three batches: heads 0-3 and 4-6 overlap attention.
        # rstd = rsqrt(ms+eps) entirely on DVE (magic seed + 2 Newton steps)
        # so ACT's activation table stays on Exp.
        def emit_rms(h_lo, h_hi):
            w = (h_hi - h_lo) * NT
            m = comb.tile([128, w], F32, tag=f"rm{h_lo}", name=f"rm{h_lo}")
            yv = comb.tile([128, w], F32, tag=f"ry{h_lo}", name=f"ry{h_lo}")
            t = comb.tile([128, w], F32, tag=f"rt{h_lo}", name=f"rt{h_lo}")
            nc.vector.tensor_scalar(
                out=m,
                in0=ssq_all[:, h_lo * NT : h_hi * NT],
                scalar1=1.0 / DV,
                scalar2=EPS,
                op0=AluOp.mult,
                op1=AluOp.add,
            )
            # seed: y = bitcast(magic - (bitcast_i32(m) >> 1))
            nc.vector.tensor_scalar(
                out=t.bitcast(I32),
                in0=m.bitcast(I32),
                scalar1=1,
                scalar2=None,
                op0=AluOp.logical_shift_right,
            )
            nc.vector.scalar_tensor_tensor(
                out=yv.bitcast(I32),
                in0=magic_sb[:].broadcast_to([128, w]).bitcast(I32),
                scalar=1,
                in1=t.bitcast(I32),
                op0=AluOp.mult,
                op1=AluOp.subtract,
            )
            for _ in range(2):  # Newton: y *= 1.5 - 0.5*m*y^2
                nc.vector.scalar_tensor_tensor(
                    out=t, in0=yv, scalar=1.0, in1=yv,
                    op0=AluOp.mult, op1=AluOp.mult,
                )
                nc.vector.scalar_tensor_tensor(
                    out=t, in0=t, scalar=0.5, in1=m,
                    op0=AluOp.mult, op1=AluOp.mult,
                )
                nc.vector.scalar_tensor_tensor(
                    out=t, in0=t, scalar=-1.0,
                    in1=c1p5_sb[:].broadcast_to([128, w]),
                    op0=AluOp.mult, op1=AluOp.add,
                )
                nc.vector.scalar_tensor_tensor(
                    out=yv, in0=yv, scalar=1.0, in1=t,
                    op0=AluOp.mult, op1=AluOp.mult,
                )
            for h in range(h_lo, h_hi):
                nc.vector.scalar_tensor_tensor(
                    out=outcat_sb[:, :, h * DV : (h + 1) * DV],
                    in0=oh_all[:, h, :, :],
                    scalar=1.0,
                    in1=_bcast(yv[:, (h - h_lo) * NT : (h - h_lo + 1) * NT], DV),
                    op0=AluOp.mult,
                    op1=AluOp.mult,
                )

        def epilogue_bank(h, b, u_tiles):
            uv = [
                u_tiles[e][b][:].rearrange("p (q d) -> p q d", q=4)
                for e in range(2)
            ]
            rr = [
                comb.tile([128, 4], F32, tag=f"rr{e}", name=f"rr{e}_{h}_{b}")
                for e in range(2)
            ]
            for e in range(2):
                nc.vector.reciprocal(out=rr[e], in_=uv[e][:, :, DV])
            t2b = comb.tile([128, 4, DV], F32, tag="t2b", name=f"t2b_{h}_{b}")
            nc.vector.scalar_tensor_tensor(
                out=t2b,
                in0=uv[1][:, :, 0:DV],
                scalar=lam_sb[:],
                in1=_bcast(rr[1], DV),
                op0=AluOp.mult,
                op1=AluOp.mult,
            )
            oh4 = oh_all[:, h, b * 4 : (b + 1) * 4, :]
            nc.vector.scalar_tensor_tensor(
                out=oh4,
                in0=uv[0][:, :, 0:DV],
                scalar=1.0,
                in1=_bcast(rr[0], DV),
                op0=AluOp.mult,
                op1=AluOp.mult,
            )
            nc.vector.scalar_tensor_tensor(
                out=oh4,
                in0=oh4,
                scalar=1.0,
                in1=t2b,
                op0=AluOp.mult,
                op1=AluOp.subtract,
            )
            for tj in range(b * 4, (b + 1) * 4):
                sq = comb.tile([128, DV], F32, tag="sq", name=f"sq_{h}_{tj}")
                nc.vector.scalar_tensor_tensor(
                    out=sq,
                    in0=oh_all[:, h, tj, :],
                    scalar=1.0,
                    in1=oh_all[:, h, tj, :],
                    op0=AluOp.mult,
                    op1=AluOp.mult,
                    accum_out=ssq_all[:, h * NT + tj : h * NT + tj + 1],
                )

        def rms_last_half(b):
            # rsqrt + outcat for the last head, one u-bank at a time (bank 0
            # runs mid-loop so only bank 1 sits on the tail critical path)
            h = HPG - 1
            w = 4
            m = comb.tile([128, w], F32, tag=f"rl{b}", name=f"rl{b}")
            yv = comb.tile([128, w], F32, tag=f"ryl{b}", name=f"ryl{b}")
            t = comb.tile([128, w], F32, tag=f"rtl{b}", name=f"rtl{b}")
            nc.vector.tensor_scalar(
                out=m,
                in0=ssq_all[:, h * NT + b * 4 : h * NT + (b + 1) * 4],
                scalar1=1.0 / DV,
                scalar2=EPS,
                op0=AluOp.mult,
                op1=AluOp.add,
            )
            nc.vector.tensor_scalar(
                out=t.bitcast(I32),
                in0=m.bitcast(I32),
                scalar1=1,
                scalar2=None,
                op0=AluOp.logical_shift_right,
            )
            nc.vector.scalar_tensor_tensor(
                out=yv.bitcast(I32),
                in0=magic_sb[:].broadcast_to([128, w]).bitcast(I32),
                scalar=1,
                in1=t.bitcast(I32),
                op0=AluOp.mult,
                op1=AluOp.subtract,
            )
            for _ in range(2):
                nc.vector.scalar_tensor_tensor(
                    out=t, in0=yv, scalar=1.0, in1=yv,
                    op0=AluOp.mult, op1=AluOp.mult,
                )
                nc.vector.scalar_tensor_tensor(
                    out=t, in0=t, scalar=0.5, in1=m,
                    op0=AluOp.mult, op1=AluOp.mult,
                )
                nc.vector.scalar_tensor_tensor(
                    out=t, in0=t, scalar=-1.0,
                    in1=c1p5_sb[:].broadcast_to([128, w]),
                    op0=AluOp.mult, op1=AluOp.add,
                )
                nc.vector.scalar_tensor_tensor(
                    out=yv, in0=yv, scalar=1.0, in1=t,
                    op0=AluOp.mult, op1=AluOp.mult,
                )
            nc.vector.scalar_tensor_tensor(
                out=outcat_sb[:, b * 4 : (b + 1) * 4, h * DV : (h + 1) * DV],
                in0=oh_all[:, h, b * 4 : (b + 1) * 4, :],
                scalar=1.0,
                in1=_bcast(yv, DV),
                op0=AluOp.mult,
                op1=AluOp.mult,
            )

        for h in range(HPG):
            kc_ = 4 + h // 2
            s_tiles = None
            u_tiles = [
                [
                    u_pool.tile(
                        [128, 512], F32, tag=f"u{e}{b}", name=f"u{e}{b}_{h}"
                    )
                    for b in range(2)
                ]
                for e in range(2)
            ]
            pb = prebaked.get(h, [])
            if pb:
                for ps_, pes_, pch_ in pb[:-1]:
                    emit_av(h, ps_, 0, pes_[0][:], pch_)
                    emit_av(h, ps_, 1, pes_[1][:], pch_)
                prev = pb[-1]
                s_start = len(pb)
            else:
                prev = None
                s_start = 0
            if h == 0:
                # live iters 2-3 behind h0's dump so ACT stays fed
                emit_live_iter(1)
                emit_live_iter(2)
            elif h == 1:
                emit_live_iter(3)
            if h == h_live:
                s_tiles = live_s_tiles
            for s in range(s_start, NT):
                t0 = 128 * s
                chunks = [(t0, 512), (512, 1024)] if s < 4 else [(t0, 1024)]
                if s_tiles is None:
                    s_tiles = [
                        s_pool.tile([128, T], F32, tag=f"s{e}", name=f"s{e}_{h}")
                        for e in range(2)
                    ]
                es2 = [
                    es_pool.tile([128, T], F16, tag=f"es{e}", name=f"es{e}_{h}_{s}")
                    for e in range(2)
                ]
                for e in range(2):
                    for c0, c1 in chunks:
                        nc.tensor.matmul(
                            s_tiles[e][:, c0:c1],
                            qkT_sb[:, kc_, t0 : t0 + 128],
                            qTp_sb[:, 2 * h + e, c0:c1],
                            start=True,
                            stop=True,
                        )
                if prev is not None:
                    ps_, pes_, pchunks_ = prev
                    emit_av(h, ps_, 0, pes_[0][:], pchunks_)
                    emit_av(h, ps_, 1, pes_[1][:], pchunks_)
                if h == HPG - 1 and s == 5:
                    # u-bank 0 is final after the s=4 flush: run its combine
                    # + the half-RMS now, off the tail critical path
                    epilogue_bank(h, 0, u_tiles)
                    rms_last_half(0)
                for e in range(2):
                    nc.scalar.activation(
                        out=es2[e][:, t0:T],
                        in_=s_tiles[e][:, t0:T],
                        func=Act.Exp,
                        scale=1.0 / 32.0,
                    )
                    # causal mask inside the diagonal block: keep t >= s
                    nc.gpsimd.affine_select(
                        out=es2[e][:, t0 : t0 + 128],
                        in_=es2[e][:, t0 : t0 + 128],
                        pattern=[[1, 128]],
                        compare_op=AluOp.is_ge,
                        fill=0.0,
                        base=0,
                        channel_multiplier=-1,
                    )
                prev = (s, es2, chunks)
            ps_, pes_, pchunks_ = prev
            emit_av(h, ps_, 0, pes_[0][:], pchunks_)
            emit_av(h, ps_, 1, pes_[1][:], pchunks_)

            if h == HPG - 1:
                epilogue_bank(h, 1, u_tiles)
                rms_last_half(1)
            else:
                for b in range(2):
                    epilogue_bank(h, b, u_tiles)
            if h == 3:
                emit_rms(0, 4)
            elif h == 6:
                emit_rms(4, 7)

    # ---------------- phase 4+5: PE transpose + output projection ----------------
    with (
        tc.tile_pool(name="tps", bufs=3, space="PSUM") as tps,
        tc.tile_pool(name="pps", bufs=4, space="PSUM") as pps,
        tc.tile_pool(name="yout", bufs=2) as yout,
    ):
        def emit_transpose(tj):
            tp = tps.tile([128, 4, 128], F16, tag="tp", name=f"tp{tj}")
            for rrb in range(4):
                nc.tensor.transpose(
                    tp[:, rrb, :],
                    outcat_sb[:, tj, rrb * 128 : (rrb + 1) * 128],
                    ident[:],
                )
            if tj % 2 == 0:
                nc.vector.tensor_copy(
                    out=outcatT_sb[:, :, tj * 128 : (tj + 1) * 128], in_=tp
                )
            else:
                nc.scalar.copy(
                    out=outcatT_sb[:, :, tj * 128 : (tj + 1) * 128], in_=tp
                )

        emit_transpose(0)
        emit_transpose(1)
        for tt in range(NT):
            if tt + 2 < NT:
                emit_transpose(tt + 2)
            yt = yout.tile([128, C], F16, tag="yt", name=f"y{tt}")
            for nh in range(2):
                ps = pps.tile([128, 512], F32, tag="pp", name=f"pp{tt}{nh}")
                for rr_ in range(4):
                    nc.tensor.matmul(
                        ps,
                        outcatT_sb[:, rr_, tt * 128 : (tt + 1) * 128],
                        wp_sb[:, rr_, nh * 512 : (nh + 1) * 512],
                        start=(rr_ == 0),
                        stop=(rr_ == 3),
                    )
                if nh == 0:
                    nc.vector.tensor_copy(
                        out=yt[:, nh * 512 : (nh + 1) * 512], in_=ps
                    )
                else:
                    nc.scalar.copy(out=yt[:, nh * 512 : (nh + 1) * 512], in_=ps)
                nc.sync.dma_start(
                    out=y[tt * 128 : (tt + 1) * 128, nh * 512 : (nh + 1) * 512],
                    in_=yt[:, nh * 512 : (nh + 1) * 512],
                )


def build_nc():
    nc = bass.Bass()
    xT = nc.declare_dram_parameter("xT", [C, T], F16, isOutput=False)
    w_qk = nc.declare_dram_parameter("w_qk", [C, COLS], F16, isOutput=False)
    w_v = nc.declare_dram_parameter("w_v", [C, 512], F16, isOutput=False)
    w_p = nc.declare_dram_parameter("w_p", [512, C], F16, isOutput=False)
    lam = nc.declare_dram_parameter("lam", [128, 1], F32, isOutput=False)
    y = nc.declare_dram_parameter("y", [T, C], F16, isOutput=True)
    with tile.TileContext(nc) as tc:
        with ExitStack() as ctx:
            _emit(ctx, tc, xT, w_qk, w_v, w_p, lam, y)
    return nc


_NC = None


def _get_nc():
    global _NC
    if _NC is None:
        _NC = build_nc()
    return _NC


def make_in_maps(x, w_attn, w_proj, lambda_q1, lambda_q2, lambda_k1, lambda_k2, gamma):
    x = np.asarray(x, np.float32)
    w_attn = np.asarray(w_attn, np.float32)
    w_proj = np.asarray(w_proj, np.float32)
    lam1 = np.exp(np.sum(np.float32(lambda_q1) * np.float32(lambda_k1), dtype=np.float32))
    lam2 = np.exp(np.sum(np.float32(lambda_q2) * np.float32(lambda_k2), dtype=np.float32))
    lam_full = np.float32(lam1 - lam2 + LAMBDA_INIT)
    lam_tile = np.full((128, 1), lam_full, np.float32)
    # fold gamma * (1 - lambda_init) into w_proj rows
    scale = np.tile(np.asarray(gamma, np.float32), H_TOT) * np.float32(1.0 - LAMBDA_INIT)
    w_p_full = (w_proj * scale[:, None]).astype(np.float16)

    in_maps = []
    for core in range(N_CORES):
        b, g = core // G, core % G
        in_maps.append(
            {
                "xT": np.ascontiguousarray(x[b].T.astype(np.float16)),
                "w_qk": np.ascontiguousarray(
                    np.concatenate(
                        [
                            w_attn[:, g * 512 : (g + 1) * 512],
                            w_attn[:, C + g * 512 : C + (g + 1) * 512],
                        ],
                        axis=1,
                    ).astype(np.float16)
                ),
                "w_v": np.ascontiguousarray(
                    w_attn[:, 2 * C + g * 512 : 2 * C + (g + 1) * 512].astype(
                        np.float16
                    )
                ),
                "w_p": np.ascontiguousarray(w_p_full[g * 512 : (g + 1) * 512, :]),
                "lam": lam_tile,
            }
        )
    return in_maps


def assemble(results):
    y = np.empty((B, T, C), np.float32)
    for b in range(B):
        y[b] = results[b * G]["y"].astype(np.float32) + results[b * G + 1][
            "y"
        ].astype(np.float32)
    return y


def kernel(**inputs) -> np.ndarray:
    nc = _get_nc()
    in_maps = make_in_maps(**inputs)
    res = run_bass_kernel_spmd(nc, in_maps, list(range(N_CORES)))
    return assemble(res.results)


# revision 12
# speedup vs baseline: 1.0519x; 1.0519x over previous
"""MultiHeadDiffAttn Trainium2 kernel (v6, per-half-head PSUM/es tiles).

Sharding: 8 cores = 4-way data parallel over batch x 2-way tensor parallel
over heads (8 v-heads / 16 half-heads per core).  Each core computes its
batch's qkv projection restricted to its head group, differential attention
with per-half-head softmax, head RMS norm, and a partial output projection
(its 512 rows of w_proj).  Host sums the two partial projections per batch.

Measured device behavior this kernel is shaped around:
  - fp32 matmul streams at ~1/4 the 16-bit rate, so all matmul operands are
    fp16 (PSUM accumulation stays fp32).
  - K<128 16-bit matmuls stream at 2 cycles/col, so the S^T matmuls pad the
    contraction to K=128 via the zero-padded qTp buffer.
  - ACT exp is the attention-phase bottleneck (~0.83ns/col + ~200ns/op).
    Each half-head keeps its OWN S-psum tile and its own es tile: sharing
    one [128,2,T] tile between the pair made S(s+1) wait on BOTH exps of
    iteration s (the dep tracker does not split the halves), serializing
    the loop (v3/v5 lost 30-45us to that).  S matmuls are emitted e-outer
    so exp(e0) starts after half the S work; the causal mask runs per
    half-head on GpSimd; the diagonal AV is emitted LAST per (s,e) group
    so the mask latency hides behind the off-diagonal AVs.
  - 12 s-iterations (all of head 0 + head 1's first 4) are prebaked into
    the tensor-bound qkv phase so ACT works from ~20us: cc order
    0,4,1,5,2,6,3,7 makes the needed chunks finish first, and the head-1
    prebakes interleave between v-projection t-blocks.
  - qTp scatter stays on DVE ([32,*] ops on GpSimd use 2 of 8 Q7 cores and
    run ~4x slow — v4 lost 60us to that).
  - RMS rstd = rsqrt(ms+eps) is computed entirely on DVE (bit-trick seed +
    2 Newton steps on int32/f32 bitcast views): keeping Sqrt off ACT avoids
    the Exp<->Sqrt activation-table thrash (~1.3us per reload).
  - Epilogue runs wide: batched reciprocals [128,4] over the den columns,
    lambda folded via the scalar slot, bank-wide [128,4,64] ops with
    stride-0 broadcast APs, one outcat scale op per head.  RMS in three
    batches (0-3 / 4-6 / 7) so epilogue overlaps attention.
  - y is stored f16; the tail yt copies run on the (idle-by-then) ACT.
"""

import math
from contextlib import ExitStack

import numpy as np

import concourse.bass as bass
import concourse.tile as tile
from concourse import masks, mybir
from concourse.bass_utils import run_bass_kernel_spmd

# The deployed walrus rejects instructions carrying more than one sync wait
# ("Too many sync wait commands" in setupSyncWait).  Legalize at the BIR-JSON
# level: for every instruction with >1 wait, hoist the extra waits onto NoOp
# instructions inserted just before it on the same engine (engine streams are
# in-order, so semantics are identical).
_MAX_WAITS = 1


def _legalize_sync_waits(d):
    for f in d.get("functions", []):
        for bb in f.get("blocks", []):
            out = []
            for inst in bb["instructions"]:
                si = inst.get("sync_info")
                waits = (si or {}).get("on_wait") or []
                if len(waits) > _MAX_WAITS:
                    extra = waits[: len(waits) - _MAX_WAITS]
                    keep = waits[len(waits) - _MAX_WAITS :]
                    for j in range(0, len(extra), _MAX_WAITS):
                        nop = {
                            "engine": inst["engine"],
                            "ins": [],
                            "outs": [],
                            "name": f"{inst['name']}-lw{j}",
                            "opcode": "NoOp",
                            "sync_info": {
                                "on_wait": extra[j : j + _MAX_WAITS],
                                "on_update": [],
                            },
                        }
                        if "debug" in inst:
                            nop["debug"] = inst["debug"]
                        out.append(nop)
                    si["on_wait"] = keep
                out.append(inst)
            bb["instructions"] = out
    return d


_orig_to_json_bytes = bass.Bass.to_json_bytes


def _patched_to_json_bytes(self, *a, **kw):
    import json as _json

    raw = _orig_to_json_bytes(self, *a, **kw)
    return _json.dumps(_legalize_sync_waits(_json.loads(raw))).encode()


bass.Bass.to_json_bytes = _patched_to_json_bytes

F32 = mybir.dt.float32
F16 = mybir.dt.float16
I32 = mybir.dt.int32

B, T, C = 4, 1024, 1024
H_TOT = 16  # total v-heads
HD = 32  # half-head dim
DV = 64  # v-head dim
G = 2  # head groups (tensor parallel)
HPG = H_TOT // G  # 8 v-heads per core
COLS = 1024  # q cols + k cols per group
LAMBDA_INIT = 0.8 - 0.6 * math.exp(-0.3 * (1 - 1))  # 0.2
EPS = 1e-5
N_CORES = 8

NT = T // 128  # 8 t-tiles
NKC = C // 128  # 8 contraction chunks
RSQRT_MAGIC = 0x5F3759DF


def _bcast(ap, n):
    """[128, m] -> [128, m, n] with stride-0 last dim."""
    return ap.unsqueeze(2).broadcast_to([ap.shape[0], ap.shape[1], n])


def _emit(ctx: ExitStack, tc: tile.TileContext, xT, w_qk, w_v, w_p, lam, y):
    nc = tc.nc
    AluOp = mybir.AluOpType
    Act = mybir.ActivationFunctionType

    const = ctx.enter_context(tc.tile_pool(name="const", bufs=1))
    ident = const.tile([128, 128], F16)
    masks.make_identity(nc, ident[:])
    lam_sb = const.tile([128, 1], F32)
    nc.sync.dma_start(out=lam_sb, in_=lam[:])
    magic_sb = const.tile([128, 1], I32)
    nc.vector.memset(magic_sb, RSQRT_MAGIC)
    c1p5_sb = const.tile([128, 1], F32)
    nc.vector.memset(c1p5_sb, 1.5)

    big = ctx.enter_context(tc.tile_pool(name="big", bufs=1))
    qkT_sb = big.tile([128, 8, T], F16)  # row-chunks of [COLS, T]
    v_sb = big.tile([128, NT, HPG, 128], F16)  # [s-chunk][head][dv | ones | 0-pad]
    wp_sb = big.tile([128, 4, C], F16)
    # per-half-head q, zero-padded to K=128: data lives at the same 32-row
    # strip as that half-head's k rows inside its qkT chunk, so the S^T
    # matmul can contract over the full 128 partitions at full stream rate
    # (the other half-heads' k rows meet zero q rows).
    qTp_sb = big.tile([128, 2 * HPG, T], F16)

    # zero qTp per half-head group so each scatter only waits on its own
    # memset; then the v padding
    for cc in range(4):
        nc.gpsimd.memset(qTp_sb[:, 4 * cc : 4 * cc + 4, :], 0.0)
    nc.gpsimd.memset(v_sb[:, :, :, DV + 1 :], 0.0)
    es_pool = ctx.enter_context(tc.tile_pool(name="es", bufs=19))

    prebaked = {}  # h -> list of (s, es2, chunks)

    # ---------------- phase 1+2: qkv projections + prebaked attention ----------------
    with (
        tc.tile_pool(name="xw", bufs=1) as xw,
        tc.tile_pool(name="mmps", bufs=3, space="PSUM") as mmps,
        tc.tile_pool(name="spre", bufs=1, space="PSUM") as spre,
    ):
        xT_sb = xw.tile([128, NKC, T], F16)
        wqk_sb = xw.tile([128, NKC, COLS], F16)
        wv_sb = xw.tile([128, NKC, 512], F16)

        xT_r = xT[:].rearrange("(c p) t -> p c t", p=128)
        wqk_r = w_qk[:].rearrange("(c p) m -> p c m", p=128)

        def load_wqk(cc):
            nc.sync.dma_start(
                out=wqk_sb[:, :, cc * 128 : (cc + 1) * 128],
                in_=wqk_r[:, :, cc * 128 : (cc + 1) * 128],
            )

        # load order: full 2KB-run rows per kc-chunk (a 128-col slice makes
        # 256B DMA runs, descriptor-latency-bound; full rows are 4-8x denser),
        # wqk and xT interleaved kc-by-kc in matmul consumption order
        for kc in range(NKC):
            nc.sync.dma_start(out=wqk_sb[:, kc, :], in_=wqk_r[:, kc, :])
            nc.sync.dma_start(out=xT_sb[:, kc, :], in_=xT_r[:, kc, :])
        nc.sync.dma_start(out=wv_sb, in_=w_v[:].rearrange("(c p) m -> p c m", p=128))
        nc.sync.dma_start(
            out=wp_sb, in_=w_p[:].rearrange("(c p) m -> p c m", p=128)
        )

        def emit_prebake(h, s):
            t0 = 128 * s
            chunks = [(t0, 512), (512, 1024)] if s < 4 else [(t0, 1024)]
            s2 = [
                spre.tile([128, T], F32, tag=f"spre{e}", name=f"sp{e}_{h}_{s}")
                for e in range(2)
            ]
            es2 = [
                es_pool.tile([128, T], F16, tag=f"es{e}", name=f"esp{e}_{h}_{s}")
                for e in range(2)
            ]
            for e in range(2):
                for c0, c1 in chunks:
                    nc.tensor.matmul(
                        s2[e][:, c0:c1],
                        qkT_sb[:, 4 + h // 2, t0 : t0 + 128],
                        qTp_sb[:, 2 * h + e, c0:c1],
                        start=True,
                        stop=True,
                    )
                nc.scalar.activation(
                    out=es2[e][:, t0:T],
                    in_=s2[e][:, t0:T],
                    func=Act.Exp,
                    scale=1.0 / 32.0,
                )
                nc.gpsimd.affine_select(
                    out=es2[e][:, t0 : t0 + 128],
                    in_=es2[e][:, t0 : t0 + 128],
                    pattern=[[1, 128]],
                    compare_op=AluOp.is_ge,
                    fill=0.0,
                    base=0,
                    channel_multiplier=-1,
                )
            prebaked.setdefault(h, []).append((s, es2, chunks))

        # qkT[cc-block, :] = w_qk[:, cc-block].T @ x^T
        # Interleaved cc order so the prebakes (need the cc=0/1 scatters
        # + the cc=4 kT chunk) can start early.
        cc_order = [0, 4, 1, 5, 2, 6, 3, 7]
        prebake_after = {1: (0, 0), 2: (0, 1), 3: (0, 2), 4: (0, 3),
                         5: (0, 4), 6: (0, 5), 7: (0, 6)}  # h0 s7 after loop
        for pos, cc in enumerate(cc_order):
            for nh in range(2):
                ps = mmps.tile([128, 512], F32, tag="mmps", name=f"qk{cc}{nh}")
                for kc in range(NKC):
                    nc.tensor.matmul(
                        ps,
                        wqk_sb[:, kc, cc * 128 : (cc + 1) * 128],
                        xT_sb[:, kc, nh * 512 : (nh + 1) * 512],
                        start=(kc == 0),
                        stop=(kc == NKC - 1),
                    )
                nc.vector.tensor_copy(
                    out=qkT_sb[:, cc, nh * 512 : (nh + 1) * 512], in_=ps
                )
                if cc < 4:  # q chunk: scatter the 4 half-heads into qTp
                    for j in range(4):
                        hh = cc * 4 + j
                        nc.vector.tensor_copy(
                            out=qTp_sb[
                                j * 32 : (j + 1) * 32,
                                hh,
                                nh * 512 : (nh + 1) * 512,
                            ],
                            in_=qkT_sb[
                                j * 32 : (j + 1) * 32,
                                cc,
                                nh * 512 : (nh + 1) * 512,
                            ],
                        )
            if pos in prebake_after:
                emit_prebake(*prebake_after[pos])
        emit_prebake(0, 7)

        # v[t-block, :] = x @ w_v ; scatter heads into v_sb, slot 64 = ones;
        # head-1 prebakes interleave between the t-blocks
        for tt in range(NT):
            ps = mmps.tile([128, 512], F32, tag="mmps", name=f"v{tt}")
            for kc in range(NKC):
                nc.tensor.matmul(
                    ps,
                    xT_sb[:, kc, tt * 128 : (tt + 1) * 128],
                    wv_sb[:, kc, :],
                    start=(kc == 0),
                    stop=(kc == NKC - 1),
                )
            nc.vector.tensor_copy(
                out=v_sb[:, tt, :, 0:DV],
                in_=ps[:].rearrange("p (h d) -> p h d", h=HPG),
            )
            nc.vector.memset(v_sb[:, tt, :, DV : DV + 1], 1.0)
            emit_prebake(1, tt)

    mid = ctx.enter_context(tc.tile_pool(name="mid", bufs=1))
    outcat_sb = mid.tile([128, NT, HPG * DV], F16)  # [t-chunk][512]
    outcatT_sb = mid.tile([128, 4, T], F16)  # row-chunks of [512, T]

    # ---------------- phase 3: differential attention ----------------
    with (
        tc.tile_pool(name="sps", bufs=1, space="PSUM") as s_pool,
        tc.tile_pool(name="ups", bufs=1, space="PSUM") as u_pool,
        tc.tile_pool(name="comb", bufs=6) as comb,
        tc.tile_pool(name="ohp", bufs=1, space="SBUF") as ohp,
    ):
        oh_all = ohp.tile([128, HPG, NT, DV], F32, tag="ohall", name="ohall")
        ssq_all = ohp.tile([128, HPG * NT], F32, tag="ssqall", name="ssqall")

        # first live head's opening iterations, emitted around the prebaked
        # heads' AV dumps so ACT has work while the dumps drain
        h_live = max(prebaked.keys()) + 1
        live_s_tiles = [
            s_pool.tile([128, T], F32, tag=f"s{e}", name=f"s{e}_{h_live}")
            for e in range(2)
        ]

        def emit_live_iter(s):
            t0 = 128 * s
            chunks = [(t0, 512), (512, 1024)] if s < 4 else [(t0, 1024)]
            es2 = [
                es_pool.tile(
                    [128, T], F16, tag=f"es{e}", name=f"es{e}_{h_live}_{s}"
                )
                for e in range(2)
            ]
            for e in range(2):
                for c0, c1 in chunks:
                    nc.tensor.matmul(
                        live_s_tiles[e][:, c0:c1],
                        qkT_sb[:, 4 + h_live // 2, t0 : t0 + 128],
                        qTp_sb[:, 2 * h_live + e, c0:c1],
                        start=True,
                        stop=True,
                    )
                nc.scalar.activation(
                    out=es2[e][:, t0:T],
                    in_=live_s_tiles[e][:, t0:T],
                    func=Act.Exp,
                    scale=1.0 / 32.0,
                )
                nc.gpsimd.affine_select(
                    out=es2[e][:, t0 : t0 + 128],
                    in_=es2[e][:, t0 : t0 + 128],
                    pattern=[[1, 128]],
                    compare_op=AluOp.is_ge,
                    fill=0.0,
                    base=0,
                    channel_multiplier=-1,
                )
            prebaked.setdefault(h_live, []).append((s, es2, chunks))

        emit_live_iter(0)

        def emit_av(h, s, e, es_e, chunks):
            # U[t-block, dv|den] += expS^T[s-block, t-block].T @ v_aug[s-block]
            # diag tj last so the fused-mask latency hides behind the others;
            # start fires on the first tj EMITTED for each bank at s==0
            tjs = list(range(s + 1, NT)) + [s]
            for tj in tjs:
                off = (tj % 4) * 128
                nc.tensor.matmul(
                    u_tiles[e][tj // 4][:, off : off + DV + 1],
                    es_e[:, tj * 128 : (tj + 1) * 128],
                    v_sb[:, s, h, 0 : DV + 1],
                    start=(s == 0 and tj in (1, 4)),
                    stop=(s == tj and tj % 4 == 3),
                )

        # ---- RMS in three batches: heads 0-3 and 4-6 overlap attention.
        # rstd = rsqrt(ms+eps) entirely on DVE (magic seed + 2 Newton steps)
        # so ACT's activation table stays on Exp.
        def emit_rms(h_lo, h_hi):
            w = (h_hi - h_lo) * NT
            m = comb.tile([128, w], F32, tag=f"rm{h_lo}", name=f"rm{h_lo}")
            yv = comb.tile([128, w], F32, tag=f"ry{h_lo}", name=f"ry{h_lo}")
            t = comb.tile([128, w], F32, tag=f"rt{h_lo}", name=f"rt{h_lo}")
            nc.vector.tensor_scalar(
                out=m,
                in0=ssq_all[:, h_lo * NT : h_hi * NT],
                scalar1=1.0 / DV,
                scalar2=EPS,
                op0=AluOp.mult,
                op1=AluOp.add,
            )
            # seed: y = bitcast(magic - (bitcast_i32(m) >> 1))
            nc.vector.tensor_scalar(
                out=t.bitcast(I32),
                in0=m.bitcast(I32),
                scalar1=1,
                scalar2=None,
                op0=AluOp.logical_shift_right,
            )
            nc.vector.scalar_tensor_tensor(
                out=yv.bitcast(I32),
                in0=magic_sb[:].broadcast_to([128, w]).bitcast(I32),
                scalar=1,
                in1=t.bitcast(I32),
                op0=AluOp.mult,
                op1=AluOp.subtract,
            )
            for _ in range(2):  # Newton: y *= 1.5 - 0.5*m*y^2
                nc.vector.scalar_tensor_tensor(
                    out=t, in0=yv, scalar=1.0, in1=yv,
                    op0=AluOp.mult, op1=AluOp.mult,
                )
                nc.vector.scalar_tensor_tensor(
                    out=t, in0=t, scalar=0.5, in1=m,
                    op0=AluOp.mult, op1=AluOp.mult,
                )
                nc.vector.scalar_tensor_tensor(
                    out=t, in0=t, scalar=-1.0,
                    in1=c1p5_sb[:].broadcast_to([128, w]),
                    op0=AluOp.mult, op1=AluOp.add,
                )
                nc.vector.scalar_tensor_tensor(
                    out=yv, in0=yv, scalar=1.0, in1=t,
                    op0=AluOp.mult, op1=AluOp.mult,
                )
            for h in range(h_lo, h_hi):
                nc.vector.scalar_tensor_tensor(
                    out=outcat_sb[:, :, h * DV : (h + 1) * DV],
                    in0=oh_all[:, h, :, :],
                    scalar=1.0,
                    in1=_bcast(yv[:, (h - h_lo) * NT : (h - h_lo + 1) * NT], DV),
                    op0=AluOp.mult,
                    op1=AluOp.mult,
                )

        def epilogue_bank(h, b, u_tiles):
            uv = [
                u_tiles[e][b][:].rearrange("p (q d) -> p q d", q=4)
                for e in range(2)
            ]
            rr = [
                comb.tile([128, 4], F32, tag=f"rr{e}", name=f"rr{e}_{h}_{b}")
                for e in range(2)
            ]
            for e in range(2):
                nc.vector.reciprocal(out=rr[e], in_=uv[e][:, :, DV])
            t2b = comb.tile([128, 4, DV], F32, tag="t2b", name=f"t2b_{h}_{b}")
            nc.vector.scalar_tensor_tensor(
                out=t2b,
                in0=uv[1][:, :, 0:DV],
                scalar=lam_sb[:],
                in1=_bcast(rr[1], DV),
                op0=AluOp.mult,
                op1=AluOp.mult,
            )
            oh4 = oh_all[:, h, b * 4 : (b + 1) * 4, :]
            nc.vector.scalar_tensor_tensor(
                out=oh4,
                in0=uv[0][:, :, 0:DV],
                scalar=1.0,
                in1=_bcast(rr[0], DV),
                op0=AluOp.mult,
                op1=AluOp.mult,
            )
            nc.vector.scalar_tensor_tensor(
                out=oh4,
                in0=oh4,
                scalar=1.0,
                in1=t2b,
                op0=AluOp.mult,
                op1=AluOp.subtract,
            )
            for tj in range(b * 4, (b + 1) * 4):
                sq = comb.tile([128, DV], F32, tag="sq", name=f"sq_{h}_{tj}")
                nc.vector.scalar_tensor_tensor(
                    out=sq,
                    in0=oh_all[:, h, tj, :],
                    scalar=1.0,
                    in1=oh_all[:, h, tj, :],
                    op0=AluOp.mult,
                    op1=AluOp.mult,
                    accum_out=ssq_all[:, h * NT + tj : h * NT + tj + 1],
                )

        def rms_last_half(b):
            # rsqrt + outcat for the last head, one u-bank at a time (bank 0
            # runs mid-loop so only bank 1 sits on the tail critical path)
            h = HPG - 1
            w = 4
            m = comb.tile([128, w], F32, tag=f"rl{b}", name=f"rl{b}")
            yv = comb.tile([128, w], F32, tag=f"ryl{b}", name=f"ryl{b}")
            t = comb.tile([128, w], F32, tag=f"rtl{b}", name=f"rtl{b}")
            nc.vector.tensor_scalar(
                out=m,
                in0=ssq_all[:, h * NT + b * 4 : h * NT + (b + 1) * 4],
                scalar1=1.0 / DV,
                scalar2=EPS,
                op0=AluOp.mult,
                op1=AluOp.add,
            )
            nc.vector.tensor_scalar(
                out=t.bitcast(I32),
                in0=m.bitcast(I32),
                scalar1=1,
                scalar2=None,
                op0=AluOp.logical_shift_right,
            )
            nc.vector.scalar_tensor_tensor(
                out=yv.bitcast(I32),
                in0=magic_sb[:].broadcast_to([128, w]).bitcast(I32),
                scalar=1,
                in1=t.bitcast(I32),
                op0=AluOp.mult,
                op1=AluOp.subtract,
            )
            for _ in range(2):
                nc.vector.scalar_tensor_tensor(
                    out=t, in0=yv, scalar=1.0, in1=yv,
                    op0=AluOp.mult, op1=AluOp.mult,
                )
                nc.vector.scalar_tensor_tensor(
                    out=t, in0=t, scalar=0.5, in1=m,
                    op0=AluOp.mult, op1=AluOp.mult,
                )
                nc.vector.scalar_tensor_tensor(
                    out=t, in0=t, scalar=-1.0,
                    in1=c1p5_sb[:].broadcast_to([128, w]),
                    op0=AluOp.mult, op1=AluOp.add,
                )
                nc.vector.scalar_tensor_tensor(
                    out=yv, in0=yv, scalar=1.0, in1=t,
                    op0=AluOp.mult, op1=AluOp.mult,
                )
            nc.vector.scalar_tensor_tensor(
                out=outcat_sb[:, b * 4 : (b + 1) * 4, h * DV : (h + 1) * DV],
                in0=oh_all[:, h, b * 4 : (b + 1) * 4, :],
                scalar=1.0,
                in1=_bcast(yv, DV),
                op0=AluOp.mult,
                op1=AluOp.mult,
            )

        for h in range(HPG):
            kc_ = 4 + h // 2
            s_tiles = None
            u_tiles = [
                [
                    u_pool.tile(
                        [128, 512], F32, tag=f"u{e}{b}", name=f"u{e}{b}_{h}"
                    )
                    for b in range(2)
                ]
                for e in range(2)
            ]
            pb = prebaked.get(h, [])
            if pb:
                for ps_, pes_, pch_ in pb[:-1]:
                    emit_av(h, ps_, 0, pes_[0][:], pch_)
                    emit_av(h, ps_, 1, pes_[1][:], pch_)
                prev = pb[-1]
                s_start = len(pb)
            else:
                prev = None
                s_start = 0
            if h == 0:
                emit_live_iter(1)  # second live iter behind h0's dump
            if h == h_live:
                s_tiles = live_s_tiles
            for s in range(s_start, NT):
                t0 = 128 * s
                chunks = [(t0, 512), (512, 1024)] if s < 4 else [(t0, 1024)]
                if s_tiles is None:
                    s_tiles = [
                        s_pool.tile([128, T], F32, tag=f"s{e}", name=f"s{e}_{h}")
                        for e in range(2)
                    ]
                es2 = [
                    es_pool.tile([128, T], F16, tag=f"es{e}", name=f"es{e}_{h}_{s}")
                    for e in range(2)
                ]
                for e in range(2):
                    for c0, c1 in chunks:
                        nc.tensor.matmul(
                            s_tiles[e][:, c0:c1],
                            qkT_sb[:, kc_, t0 : t0 + 128],
                            qTp_sb[:, 2 * h + e, c0:c1],
                            start=True,
                            stop=True,
                        )
                if prev is not None:
                    ps_, pes_, pchunks_ = prev
                    emit_av(h, ps_, 0, pes_[0][:], pchunks_)
                    emit_av(h, ps_, 1, pes_[1][:], pchunks_)
                if h == HPG - 1 and s == 5:
                    # u-bank 0 is final after the s=4 flush: run its combine
                    # + the half-RMS now, off the tail critical path
                    epilogue_bank(h, 0, u_tiles)
                    rms_last_half(0)
                for e in range(2):
                    nc.scalar.activation(
                        out=es2[e][:, t0:T],
                        in_=s_tiles[e][:, t0:T],
                        func=Act.Exp,
                        scale=1.0 / 32.0,
                    )
                    # causal mask inside the diagonal block: keep t >= s
                    nc.gpsimd.affine_select(
                        out=es2[e][:, t0 : t0 + 128],
                        in_=es2[e][:, t0 : t0 + 128],
                        pattern=[[1, 128]],
                        compare_op=AluOp.is_ge,
                        fill=0.0,
                        base=0,
                        channel_multiplier=-1,
                    )
                prev = (s, es2, chunks)
            ps_, pes_, pchunks_ = prev
            emit_av(h, ps_, 0, pes_[0][:], pchunks_)
            emit_av(h, ps_, 1, pes_[1][:], pchunks_)

            if h == HPG - 1:
                epilogue_bank(h, 1, u_tiles)
                rms_last_half(1)
            else:
                for b in range(2):
                    epilogue_bank(h, b, u_tiles)
            if h == 3:
                emit_rms(0, 4)
            elif h == 6:
                emit_rms(4, 7)

    # ---------------- phase 4+5: PE transpose + output projection ----------------
    with (
        tc.tile_pool(name="tps", bufs=3, space="PSUM") as tps,
        tc.tile_pool(name="pps", bufs=4, space="PSUM") as pps,
        tc.tile_pool(name="yout", bufs=2) as yout,
    ):
        def emit_transpose(tj):
            tp = tps.tile([128, 4, 128], F16, tag="tp", name=f"tp{tj}")
            for rrb in range(4):
                nc.tensor.transpose(
                    tp[:, rrb, :],
                    outcat_sb[:, tj, rrb * 128 : (rrb + 1) * 128],
                    ident[:],
                )
            if tj % 2 == 0:
                nc.vector.tensor_copy(
                    out=outcatT_sb[:, :, tj * 128 : (tj + 1) * 128], in_=tp
                )
            else:
                nc.scalar.copy(
                    out=outcatT_sb[:, :, tj * 128 : (tj + 1) * 128], in_=tp
                )

        emit_transpose(0)
        emit_transpose(1)
        for tt in range(NT):
            if tt + 2 < NT:
                emit_transpose(tt + 2)
            yt = yout.tile([128, C], F16, tag="yt", name=f"y{tt}")
            for nh in range(2):
                ps = pps.tile([128, 512], F32, tag="pp", name=f"pp{tt}{nh}")
                for rr_ in range(4):
                    nc.tensor.matmul(
                        ps,
                        outcatT_sb[:, rr_, tt * 128 : (tt + 1) * 128],
                        wp_sb[:, rr_, nh * 512 : (nh + 1) * 512],
                        start=(rr_ == 0),
                        stop=(rr_ == 3),
                    )
                if nh == 0:
                    nc.vector.tensor_copy(
                        out=yt[:, nh * 512 : (nh + 1) * 512], in_=ps
                    )
                else:
                    nc.scalar.copy(out=yt[:, nh * 512 : (nh + 1) * 512], in_=ps)
            nc.sync.dma_start(out=y[tt * 128 : (tt + 1) * 128, :], in_=yt)


def build_nc():
    nc = bass.Bass()
    xT = nc.declare_dram_parameter("xT", [C, T], F16, isOutput=False)
    w_qk = nc.declare_dram_parameter("w_qk", [C, COLS], F16, isOutput=False)
    w_v = nc.declare_dram_parameter("w_v", [C, 512], F16, isOutput=False)
    w_p = nc.declare_dram_parameter("w_p", [512, C], F16, isOutput=False)
    lam = nc.declare_dram_parameter("lam", [128, 1], F32, isOutput=False)
    y = nc.declare_dram_parameter("y", [T, C], F16, isOutput=True)
    with tile.TileContext(nc) as tc:
        with ExitStack() as ctx:
            _emit(ctx, tc, xT, w_qk, w_v, w_p, lam, y)
    return nc


_NC = None


def _get_nc():
    global _NC
    if _NC is None:
        _NC = build_nc()
    return _NC


def make_in_maps(x, w_attn, w_proj, lambda_q1, lambda_q2, lambda_k1, lambda_k2, gamma):
    x = np.asarray(x, np.float32)
    w_attn = np.asarray(w_attn, np.float32)
    w_proj = np.asarray(w_proj, np.float32)
    lam1 = np.exp(np.sum(np.float32(lambda_q1) * np.float32(lambda_k1), dtype=np.float32))
    lam2 = np.exp(np.sum(np.float32(lambda_q2) * np.float32(lambda_k2), dtype=np.float32))
    lam_full = np.float32(lam1 - lam2 + LAMBDA_INIT)
    lam_tile = np.full((128, 1), lam_full, np.float32)
    # fold gamma * (1 - lambda_init) into w_proj rows
    scale = np.tile(np.asarray(gamma, np.float32), H_TOT) * np.float32(1.0 - LAMBDA_INIT)
    w_p_full = (w_proj * scale[:, None]).astype(np.float16)

    in_maps = []
    for core in range(N_CORES):
        b, g = core // G, core % G
        in_maps.append(
            {
                "xT": np.ascontiguousarray(x[b].T.astype(np.float16)),
                "w_qk": np.ascontiguousarray(
                    np.concatenate(
                        [
                            w_attn[:, g * 512 : (g + 1) * 512],
                            w_attn[:, C + g * 512 : C + (g + 1) * 512],
                        ],
                        axis=1,
                    ).astype(np.float16)
                ),
                "w_v": np.ascontiguousarray(
                    w_attn[:, 2 * C + g * 512 : 2 * C + (g + 1) * 512].astype(
                        np.float16
                    )
                ),
                "w_p": np.ascontiguousarray(w_p_full[g * 512 : (g + 1) * 512, :]),
                "lam": lam_tile,
            }
        )
    return in_maps


def assemble(results):
    y = np.empty((B, T, C), np.float32)
    for b in range(B):
        y[b] = results[b * G]["y"].astype(np.float32) + results[b * G + 1][
            "y"
        ].astype(np.float32)
    return y


def kernel(**inputs) -> np.ndarray:
    nc = _get_nc()
    in_maps = make_in_maps(**inputs)
    res = run_bass_kernel_spmd(nc, in_maps, list(range(N_CORES)))
    return assemble(res.results)
